# revision 16
# baseline (speedup 1.0000x reference)
"""MQA attention block (B=2, N=2048, DIM=768, H=12, D=64) on 8 TRN2 NeuronCores.

Sharding: batch x query-block data parallel — core c handles batch c//4,
query rows (c%4)*512..+512. Each core computes K/V for its batch locally
(redundant but cheap), all 12 heads for its query block, and a full
[768, 512] slice of the output. No collectives.

Orientation: all tensors flow "transposed" (channels on partitions):
  QT[c,i] = Wq.T-proj, K2T[d,j] (duplicated to both partition halves),
  ST[j,i] scores -> exp on ACT -> AV via V_ext=[V|ones] giving out^T and
  row sums in one matmul; normalization = reciprocal + ones-broadcast
  matmul; odd heads shifted to partitions 64:128 via identity matmul so
  the final projection contracts K=128.
"""

import sys

for _p in ("/opt/trn_rl_repo",):
    if _p not in sys.path:
        sys.path.insert(0, _p)

import numpy as np
import ml_dtypes

BF = ml_dtypes.bfloat16

B, N, DIM = 2, 2048, 768
H, D = 12, 64
NQ = 512            # query rows per core
SCALE = D ** -0.5
NCORES = 8
FT = DIM // 128     # 6 partition tiles of the channel dim
JT = N // 128       # 16 key tiles
NJ = N // 512       # 4


def _patch_tile_drain(tile_mod):
    """This toolchain snapshot rejects >1 sync-wait per instruction at walrus
    codegen, but TileContext's tail drain stacks every outstanding sem wait
    onto a single Drain. Split them: one drain instruction per wait."""
    import bass_rust
    from concourse.vector_clock import ScopedClock

    def _drain_and_barrier(self, tick_clock, wait_clock):
        nc = self.nc
        drain_inst = nc.sync.drain()
        wait_clock.add_sem_waits(
            drain_inst.ins, ScopedClock({None: tick_clock.global_clock})
        )
        waits = list(drain_inst.ins.sync_info.on_wait)
        if len(waits) > 1:
            drain_inst.ins.sync_info = bass_rust.SyncInfo(
                on_wait=[waits[0]], on_update=[]
            )
            for w in waits[1:]:
                extra = nc.sync.drain()
                extra.ins.sync_info = bass_rust.SyncInfo(on_wait=[w], on_update=[])
        nc.all_engine_barrier()
        assert self.sems is not None
        popped = nc._tile_sem_poison_stack.pop()
        assert popped is self._sem_poison
        nc.clear_and_free_semaphores(list(self.sems.allocated().values()))
        nc.all_engine_barrier()

    tile_mod.TileContext._drain_and_barrier = _drain_and_barrier


def _split_multi_waits(nc):
    """Same toolchain limitation, applied globally: walrus rejects any
    instruction carrying >1 sync-wait. Move extra waits onto fresh NoOps
    inserted just before the instruction on the same engine (engine streams
    are in-order, so this is semantically identical)."""
    from concourse import mybir

    n = 0
    for f in nc.m.functions:
        for bb in f.blocks:
            insts = bb.instructions
            out = []
            for inst in insts:
                si = inst.sync_info
                waits = list(si.on_wait) if si is not None else []
                if len(waits) > 1:
                    for w in waits[:-1]:
                        n += 1
                        out.append(
                            mybir.InstNoOp(
                                name=f"waitsplit_{n}",
                                engine=inst.engine,
                                sync_info=mybir.SyncInfo(on_wait=[w], on_update=[]),
                                bass_nofuse=True,
                            )
                        )
                    inst.sync_info = mybir.SyncInfo(
                        on_wait=[waits[-1]], on_update=list(si.on_update)
                    )
                out.append(inst)
            insts[:] = out


def build_graph():
    import concourse.bass as bass
    import concourse.tile as tile
    from concourse import mybir

    _patch_tile_drain(tile)

    f32 = mybir.dt.float32
    bf16 = mybir.dt.bfloat16
    EXP = mybir.ActivationFunctionType.Exp

    nc = bass.Bass()
    xT_e = nc.declare_dram_parameter("xT", [DIM, N], bf16, isOutput=False)
    xq_e = nc.declare_dram_parameter("xq", [DIM, NQ], bf16, isOutput=False)
    wq_e = nc.declare_dram_parameter("wq", [DIM, DIM], bf16, isOutput=False)
    wkv_e = nc.declare_dram_parameter("wkv", [DIM, 2 * D], bf16, isOutput=False)
    wp_e = nc.declare_dram_parameter("wp", [DIM, DIM], bf16, isOutput=False)
    bias_e = nc.declare_dram_parameter("bias", [DIM, 1], f32, isOutput=False)
    out_e = nc.declare_dram_parameter("out", [DIM, NQ], f32, isOutput=True)

    with tile.TileContext(nc) as tc:
        with (
            tc.tile_pool(name="persist", bufs=1) as P,
            tc.tile_pool(name="work", bufs=2) as W,
            tc.tile_pool(name="psum", bufs=2, space="PSUM") as PS,
            tc.tile_pool(name="dram", bufs=2, space="DRAM") as DP,
        ):
            # ---------------- input loads ----------------
            xT = [P.tile([128, N], bf16, tag=f"xT{i}", name=f"xT{i}") for i in range(FT)]
            xq = [P.tile([128, NQ], bf16, tag=f"xq{i}", name=f"xq{i}") for i in range(FT)]
            wq = [P.tile([128, DIM], bf16, tag=f"wq{i}", name=f"wq{i}") for i in range(FT)]
            wkv = [P.tile([128, 2 * D], bf16, tag=f"wkv{i}", name=f"wkv{i}") for i in range(FT)]
            wp = [P.tile([64, DIM], bf16, tag=f"wp{i}", name=f"wp{i}") for i in range(H)]
            bias = [P.tile([128, 1], f32, tag=f"bias{i}", name=f"bias{i}") for i in range(FT)]
            for i in range(FT):
                s = slice(i * 128, (i + 1) * 128)
                nc.sync.dma_start(out=xq[i], in_=xq_e[s, :])
            for i in range(FT):
                s = slice(i * 128, (i + 1) * 128)
                nc.sync.dma_start(out=wkv[i], in_=wkv_e[s, :])

            ones = P.tile([128, D], f32, tag="ones", name="ones")
            nc.vector.memset(ones, 1.0)

            # ---------------- Q^T projection (xq/wq load first) ----------
            # qt[t] holds heads 2t (partitions 0:64) and 2t+1 (64:128).
            qt = [P.tile([128, NQ], bf16, tag=f"qt{t}", name=f"qt{t}") for t in range(FT)]
            for ct in range(FT):
                cs = slice(ct * 128, (ct + 1) * 128)
                for ft in range(FT):
                    nc.sync.dma_start(
                        out=wq[ft][:, cs], in_=wq_e[ft * 128 : (ft + 1) * 128, cs]
                    )
                ps_q = PS.tile([128, NQ], f32, tag="s", name="s")
                for ft in range(FT):
                    nc.tensor.matmul(
                        ps_q,
                        lhsT=wq[ft][:, cs],
                        rhs=xq[ft],
                        start=(ft == 0),
                        stop=(ft == FT - 1),
                    )
                nc.vector.tensor_copy(qt[ct], ps_q)

            # ---------------- K^T / V projections ----------------
            # xT streams in 512-column chunks; K^T and V proj consume each
            # chunk as it lands so the PE isn't idle during the load.
            # K2T[d, j]: K^T duplicated to partitions 0:64 and 64:128.
            k2t = P.tile([128, N], bf16, tag="k2t", name="k2t")
            vext = [P.tile([128, D + 1], bf16, tag=f"v{j}", name=f"v{j}") for j in range(JT)]
            for j in range(JT):
                nc.vector.memset(vext[j][:, D : D + 1], 1.0)
            for nj in range(NJ):
                cs = slice(nj * 512, (nj + 1) * 512)
                for i in range(FT):
                    nc.sync.dma_start(
                        out=xT[i][:, cs], in_=xT_e[i * 128 : (i + 1) * 128, cs]
                    )
                ps_k = PS.tile([64, 512], f32, tag="s", name="s")
                for ft in range(FT):
                    nc.tensor.matmul(
                        ps_k,
                        lhsT=wkv[ft][:, 0:D],
                        rhs=xT[ft][:, cs],
                        start=(ft == 0),
                        stop=(ft == FT - 1),
                    )
                nc.vector.tensor_copy(k2t[0:64, cs], ps_k)
                nc.sync.dma_start(out=k2t[64:128, cs], in_=k2t[0:64, cs])
                for j in range(nj * 4, nj * 4 + 4):
                    ps_v = PS.tile([128, D], f32, tag="av", name="av", bufs=4)
                    for ft in range(FT):
                        nc.tensor.matmul(
                            ps_v,
                            lhsT=xT[ft][:, j * 128 : (j + 1) * 128],
                            rhs=wkv[ft][:, D : 2 * D],
                            start=(ft == 0),
                            stop=(ft == FT - 1),
                        )
                    nc.vector.tensor_copy(vext[j][:, 0:D], ps_v)
            for h in range(H):
                nc.sync.dma_start(out=wp[h], in_=wp_e[h * D : (h + 1) * D, :])
            for i in range(FT):
                nc.sync.dma_start(out=bias[i], in_=bias_e[i * 128 : (i + 1) * 128, :])

            # ---------------- attention, head pairs ----------------
            # Per pair t: heads a=2t (partitions 0:64 of qt[t]) and b=2t+1
            # (64:128). Per j: two S matmuls (row groups 0/64) into the two
            # banks of one [128, 1024] psum tile, one exp for both; AV
            # matmuls trail the exps by 1 (head a) / 2 (head b) so the PE
            # fills ACT wait time. psum rows 0:64 = unnormalized out^T,
            # row 64 = softmax denominators (ones column of V_ext).
            # Normalization (recip -> gpsimd partition_broadcast -> mult,
            # odd heads + identity shift into rows 64:128 of their own AV
            # tile) is deferred into the NEXT pair iteration so the PE
            # never waits on the DVE reciprocal.
            outT = [P.tile([64, NQ], bf16, tag=f"o{h}", name=f"o{h}") for h in range(H)]

            def emit_norm(e):
                h, stage, bc = e
                nc.vector.tensor_mul(outT[h], stage[0:64, :], bc)

            pend = []
            for t in range(H // 2):
                es = [W.tile([128, 1024], bf16, tag=f"e{j}", name=f"e{j}") for j in range(JT)]
                ps_av_a = PS.tile([128, NQ], f32, tag="av", name="av_a", bufs=4)
                ps_av_b = PS.tile([128, NQ], f32, tag="av", name="av_b", bufs=4)
                for j in range(JT):
                    ps_s = PS.tile([128, 1024], f32, tag="s", name="s")
                    nc.tensor.matmul(
                        ps_s[:, 0:512],
                        lhsT=k2t[0:64, j * 128 : (j + 1) * 128],
                        rhs=qt[t][0:64, :],
                        start=True,
                        stop=True,
                    )
                    nc.tensor.matmul(
                        ps_s[:, 512:1024],
                        lhsT=k2t[64:128, j * 128 : (j + 1) * 128],
                        rhs=qt[t][64:128, :],
                        start=True,
                        stop=True,
                    )
                    nc.scalar.activation(out=es[j], in_=ps_s, func=EXP)
                    if j >= 1:
                        nc.tensor.matmul(
                            ps_av_a[0:65, :],
                            lhsT=vext[j - 1],
                            rhs=es[j - 1][:, 0:512],
                            start=(j == 1),
                            stop=False,
                        )
                    if j >= 2:
                        nc.tensor.matmul(
                            ps_av_b[0:65, :],
                            lhsT=vext[j - 2],
                            rhs=es[j - 2][:, 512:1024],
                            start=(j == 2),
                            stop=False,
                        )
                    if j == 3 and pend:
                        emit_norm(pend.pop(0))
                    if j == 7 and pend:
                        emit_norm(pend.pop(0))
                nc.tensor.matmul(
                    ps_av_a[0:65, :],
                    lhsT=vext[JT - 1],
                    rhs=es[JT - 1][:, 0:512],
                    start=False,
                    stop=True,
                )
                sta = W.tile([65, NQ], f32, tag="sta", name="sta", bufs=4)
                nc.vector.tensor_copy(sta, ps_av_a[0:65, :])
                rec_a = W.tile([65, NQ], f32, tag="rec_a", name="rec_a")
                nc.vector.reciprocal(rec_a[64:65, :], sta[64:65, :])
                bc_a = W.tile([64, NQ], f32, tag="bc_a", name="bc_a")
                rd_a = DP.tile([1, NQ], f32, tag="rd_a", name="rd_a")
                nc.sync.dma_start(out=rd_a, in_=rec_a[64:65, :])
                nc.sync.dma_start(
                    out=bc_a,
                    in_=bass.AP(tensor=rd_a.tensor, offset=rd_a.offset, ap=[[0, 64], rd_a.ap[-1]]),
                )
                for j in (JT - 2, JT - 1):
                    nc.tensor.matmul(
                        ps_av_b[0:65, :],
                        lhsT=vext[j],
                        rhs=es[j][:, 512:1024],
                        start=False,
                        stop=(j == JT - 1),
                    )
                stb = W.tile([65, NQ], f32, tag="stb", name="stb", bufs=4)
                nc.vector.tensor_copy(stb, ps_av_b[0:65, :])
                rec_b = W.tile([65, NQ], f32, tag="rec_b", name="rec_b")
                nc.vector.reciprocal(rec_b[64:65, :], stb[64:65, :])
                bc_b = W.tile([64, NQ], f32, tag="bc_b", name="bc_b")
                rd_b = DP.tile([1, NQ], f32, tag="rd_b", name="rd_b")
                nc.sync.dma_start(out=rd_b, in_=rec_b[64:65, :])
                nc.sync.dma_start(
                    out=bc_b,
                    in_=bass.AP(tensor=rd_b.tensor, offset=rd_b.offset, ap=[[0, 64], rd_b.ap[-1]]),
                )
                pend.append((2 * t, sta, bc_a))
                pend.append((2 * t + 1, stb, bc_b))
            for e in pend:
                emit_norm(e)

            # ---------------- output projection ----------------
            for cp in range(FT):
                ps_y = PS.tile([128, NQ], f32, tag="s", name="s")
                for h in range(H):
                    nc.tensor.matmul(
                        ps_y,
                        lhsT=wp[h][:, cp * 128 : (cp + 1) * 128],
                        rhs=outT[h],
                        start=(h == 0),
                        stop=(h == H - 1),
                    )
                y = W.tile([128, NQ], f32, tag="y", name="y")
                nc.vector.tensor_scalar_add(y, ps_y, bias[cp])
                nc.sync.dma_start(out=out_e[cp * 128 : (cp + 1) * 128, :], in_=y)

    _split_multi_waits(nc)
    return nc


def kernel(x, Wq, Wkv, Wproj, bproj, num_layer=None):
    from concourse.bass_utils import run_bass_kernel_spmd

    x = np.asarray(x, dtype=np.float32)
    Wq = np.asarray(Wq, dtype=np.float32)
    Wkv = np.asarray(Wkv, dtype=np.float32)
    Wproj = np.asarray(Wproj, dtype=np.float32)
    bproj = np.asarray(bproj, dtype=np.float32)

    wq_b = np.ascontiguousarray((Wq * SCALE).astype(BF))
    wkv_b = np.ascontiguousarray(Wkv.astype(BF))
    wp_b = np.ascontiguousarray(Wproj.astype(BF))
    bias_b = np.ascontiguousarray(bproj.reshape(DIM, 1))

    xT = [np.ascontiguousarray(x[b].T.astype(BF)) for b in range(B)]

    in_maps = []
    for c in range(NCORES):
        b, q0 = c // 4, (c % 4) * NQ
        in_maps.append(
            {
                "xT": xT[b],
                "xq": np.ascontiguousarray(xT[b][:, q0 : q0 + NQ]),
                "wq": wq_b,
                "wkv": wkv_b,
                "wp": wp_b,
                "bias": bias_b,
            }
        )

    nc = build_graph()
    res = run_bass_kernel_spmd(nc, in_maps, core_ids=list(range(NCORES)))

    out = np.empty((B, N, DIM), dtype=np.float32)
    for c in range(NCORES):
        b, q0 = c // 4, (c % 4) * NQ
        out[b, q0 : q0 + NQ, :] = res.results[c]["out"].T
    return out
